# revision 40
# baseline (speedup 1.0000x reference)
"""GenSP superpixel affinity for trn2 — Bass kernel, 4 cores batch-parallel.

Math (exact vs reference, not approximate):
- M_COEF=0: the two appended grid channels are identically zero -> dropped.
- Softmax over the 9 candidate superpixels: the per-pixel f2 term cancels
  inside softmax, so logits_k = 2*f.c_k - |c_k|^2.  Computed per 16x16
  pixel block (all 256 pixels of a block share the same 9 candidates) via
  a matmul with an appended constant channel:
      feats' = [f; 1]  (65 ch),  cent'_k = [2*c_k; -|c_k|^2]
      logits = feats'^T @ cent'.
- Invalid (border) candidates get cent' = [0; -30] -> exp(logit) ~ 1e-13,
  and the host drops them entirely when scattering, so they contribute 0.
- The dense (B, 256, 65536) output is 96.5% zeros: the device only computes
  the 9 nonzero values per pixel (A9); the host scatters them into the
  dense array.  This cuts device->host traffic ~50x (the axon tunnel at
  ~40 MB/s dominates wall clock) and kills the dense HBM write.

Device layout per core (one full batch image per core, cores 0-3):
- input  xs   (65, 65536) fp16: 64 feature rows + ones row (host-baked).
- output out9 (16, 128, 288) fp16: [block-row u][pixel-in-chunk][chunk c, k]
  chunk c = 2*bj + h (h = 8-pixel-row half of block (u, bj)),
  pixel p = 16*ii + jj (ii = image row within half, jj = col within block),
  k = 3*di + dj over the 3x3 candidate neighborhood (reference order).
- iter 0: affinity A0 for all pixels + centroid update sums via
  TensorE-transposed feature chunks; iter 1: affinity -> A9 -> DRAM.
"""

import numpy as np
from contextlib import ExitStack

B, C, H, W = 4, 64, 256, 256
SH = 16
NB = 16            # blocks per side
NS = NB * NB       # 256 superpixels
PIX = H * W        # 65536
CH = C + 1         # 65: features + ones row
NEG = -30.0        # border-candidate bias: exp(-30) ~ 9e-14 ~ 0

F16 = np.float16
# 10-bit fixed-point input quantization: x ~ S10 * (4*a + r), a int8, r uint2
# (4 packed per byte).  5/8 the upload bytes of fp16; rel_l2 ~4.8e-3 vs the
# 2e-2 gate (int8 alone measured 0.019, 12-bit 1.3e-3).
S10 = 5.6 / 511.0


def _build_nc():
    import concourse.bass as bass
    import concourse.bacc as bacc
    import concourse.tile as tile
    import concourse.mybir as mybir
    from concourse.masks import make_identity

    f16 = mybir.dt.float16
    f32 = mybir.dt.float32
    X = mybir.AxisListType.X

    # Bacc (not Bass): its finalize() runs move_matmul_waits_to_ldweights +
    # generate_event_semaphores, without which walrus rejects instructions
    # that accumulated >1 semaphore wait ("Too many sync wait commands").
    nc = bacc.Bacc("TRN2")
    i8 = mybir.dt.int8
    u8 = mybir.dt.uint8
    # Inputs are chunk-major (host-prearranged): free index = cidx*128 +
    # (16*ii+jj) with cidx = (u*16 + bj)*2 + h, so every matmul stationary
    # operand is a contiguous (65, 128) slice (walrus: stationary AP must be
    # 1-D free).  xs_a = int8 coarse plane; xs_r = uint2 residuals, crumb c
    # of byte j holds pixel c*PIX/4 + j.
    xs_a = nc.dram_tensor("xs_a", (C, PIX), i8, kind="ExternalInput")
    xs_r = nc.dram_tensor("xs_r", (C, PIX // 4), u8, kind="ExternalInput")
    # output quantized to uint8 (A*255): halves d2h bytes; rel_l2 7e-3
    out9 = nc.dram_tensor("out9", (NB, 128, 288), u8, kind="ExternalOutput")

    with ExitStack() as ctx:
        tc = ctx.enter_context(tile.TileContext(nc))
        singles = ctx.enter_context(tc.tile_pool(name="singles", bufs=1))
        ep = ctx.enter_context(tc.tile_pool(name="ep", bufs=3))
        ft = ctx.enter_context(tc.tile_pool(name="ft", bufs=6))
        pdot = ctx.enter_context(tc.tile_pool(name="pdot", bufs=2, space="PSUM"))
        ptr = ctx.enter_context(tc.tile_pool(name="ptr", bufs=2, space="PSUM"))
        pupd = ctx.enter_context(tc.tile_pool(name="pupd", bufs=2, space="PSUM"))
        pmisc = ctx.enter_context(tc.tile_pool(name="pmisc", bufs=1, space="PSUM"))

        feats = singles.tile([CH, PIX], f16)
        with tc.tile_pool(name="dq", bufs=2) as dq:
            # coarse plane: plain DMA + DVE convert-scale (the SWDGE
            # cast-during-DMA path is far slower); quarters bound SBUF use
            QS = PIX // 4
            for qq in range(4):
                a_t = dq.tile([C, QS], i8, tag="ast", bufs=1)
                nc.sync.dma_start(out=a_t[:], in_=xs_a[:, qq * QS:(qq + 1) * QS])
                nc.vector.tensor_scalar_mul(
                    feats[0:C, qq * QS:(qq + 1) * QS], a_t[:], 4.0 * S10)
            # 2-bit residuals, in segments to bound SBUF scratch
            SEG = PIX // 16
            for s in range(4):
                rp = dq.tile([C, SEG], u8, tag="rp")
                nc.sync.dma_start(out=rp[:], in_=xs_r[:, s * SEG:(s + 1) * SEG])
                for cr in range(4):
                    rn = dq.tile([C, SEG], u8, tag="rn")
                    if cr == 0:
                        nc.vector.tensor_scalar(
                            rn[:], rp[:], 3, None,
                            op0=mybir.AluOpType.bitwise_and)
                    elif cr == 3:
                        nc.vector.tensor_scalar(
                            rn[:], rp[:], 6, None,
                            op0=mybir.AluOpType.logical_shift_right)
                    else:
                        nc.vector.tensor_scalar(
                            rn[:], rp[:], 2 * cr, 3,
                            op0=mybir.AluOpType.logical_shift_right,
                            op1=mybir.AluOpType.bitwise_and)
                    rsc = dq.tile([C, SEG], f16, tag="rsc")
                    nc.vector.tensor_scalar_mul(rsc[:], rn[:], S10)
                    p0 = cr * (PIX // 4) + s * SEG
                    nc.vector.tensor_add(out=feats[0:C, p0:p0 + SEG],
                                         in0=feats[0:C, p0:p0 + SEG], in1=rsc[:])
        # two memsets: a single one gets AP-flattened to 65536 elements,
        # which overflows the 16-bit num_elem ISA field
        nc.vector.memset(feats[C:CH, 0:PIX // 2], 1.0)
        nc.vector.memset(feats[C:CH, PIX // 2:PIX], 1.0)
        feats_v = feats[:].rearrange("c (n p) -> c n p", p=128)  # (65, 512, 128)

        id65 = singles.tile([CH, CH], f16)
        make_identity(nc, id65[:])
        ones64 = singles.tile([C, 1], f32)
        nc.vector.memset(ones64[:], 1.0)
        ones1x = singles.tile([1, CH], f32)
        nc.vector.memset(ones1x[:], 1.0)

        num_sb = singles.tile([CH, NS], f32)
        nc.vector.memset(num_sb[:], 0.0)
        blocksum = singles.tile([C, NS], f32)
        cent1 = singles.tile([CH, NS], f32)
        sqc = singles.tile([C, NS], f32)
        centP = [singles.tile([CH, 18 * 18], f16, tag=f"centP{i}", name=f"centP{i}")
                 for i in range(2)]

        def chunk_ap(u, bj, h):
            # (65, 128) stationary: pixels of half h of block (u, bj)
            return feats_v[:, ((u * NB + bj) * 2 + h), :]

        # ---- init centroids: block sums via two DVE reduces
        rs1 = singles.tile([C, 2 * NS], f32)
        nc.vector.reduce_sum(rs1[:], feats_v[0:C], axis=X)   # per-chunk sums
        nc.vector.reduce_sum(blocksum[:].rearrange("c (a b) -> c a b", b=NB),
                             rs1[:].rearrange("c (n h) -> c n h", h=2), axis=X)

        def build_centP(idx, src, scale):
            # centP rows 0..63 = 2*scale*src (interior), row 64 = -scale^2*|src|^2
            cp = centP[idx]
            cpv = cp[:].rearrange("c (a b) -> c a b", b=18)
            nc.vector.memset(cp[0:C, :], 0.0)
            nc.vector.memset(cp[C:CH, :], NEG)
            nc.vector.tensor_scalar_mul(
                cpv[0:C, 1:17, 1:17],
                src[0:C, :].rearrange("c (a b) -> c a b", b=NB), 2.0 * scale)
            nc.vector.tensor_mul(sqc[:], src[0:C, :], src[0:C, :])
            c2p = pmisc.tile([1, NS], f32, tag="c2")
            nc.tensor.matmul(c2p[:], ones64[:], sqc[:], start=True, stop=True)
            nc.vector.tensor_scalar_mul(
                cpv[C:CH, 1:17, 1:17],
                c2p[:].rearrange("c (a b) -> c a b", b=NB), -(scale * scale))

        build_centP(0, blocksum[:], 1.0 / 256.0)

        # ---- iteration 0: affinity + update sums
        for u in range(NB):
            dot = pdot.tile([128, 32, 9], f32, tag="dot")
            for c in range(32):
                bj, h = c // 2, c % 2
                nc.tensor.matmul(
                    dot[:, c, :], chunk_ap(u, bj, h),
                    centP[0][:].rearrange("c (a b) -> c a b", b=18)[:, u:u + 3, bj:bj + 3],
                    start=True, stop=True)
            e = ep.tile([128, 32, 9], f16, tag="e")
            nc.scalar.activation(e[:], dot[:], mybir.ActivationFunctionType.Exp)
            den = ep.tile([128, 32], f32, tag="den")
            nc.vector.reduce_sum(den[:], e[:], axis=X)
            rden = ep.tile([128, 32], f32, tag="rden")
            nc.vector.reciprocal(rden[:], den[:])
            rd = rden[:]
            rden_bc = bass.AP(tensor=rd.tensor, offset=rd.offset,
                              ap=[rd.ap[0], rd.ap[1], [0, 9]])
            a0 = ep.tile([128, 32, 9], f16, tag="a0")
            nc.vector.tensor_mul(a0[:], e[:], rden_bc)

            upd = pupd.tile([CH, NB, 9], f32, tag="upd")
            for c in range(32):
                bj, h = c // 2, c % 2
                tr = ptr.tile([128, CH], f16, tag="tr")
                nc.tensor.transpose(tr[:], chunk_ap(u, bj, h), id65[:])
                ftc = ft.tile([128, CH], f16, tag="ftc")
                nc.vector.tensor_copy(out=ftc[:], in_=tr[:])
                nc.tensor.matmul(upd[:, bj, :], ftc[:], a0[:, c, :],
                                 start=(h == 0), stop=(h == 1))
            updv = upd[:].rearrange("s b (x y) -> s b x y", y=3)
            for dj in range(3):
                di0, di1 = (1 if u == 0 else 0), (2 if u == NB - 1 else 3)
                bj0, bj1 = (1 if dj == 0 else 0), (NB - 1 if dj == 2 else NB)
                src = updv[:, bj0:bj1, di0:di1, dj].rearrange("s b d -> s d b")
                dst = num_sb[:].rearrange("s (a b) -> s a b", b=NB)[
                    :, u - 1 + di0:u - 1 + di1, bj0 - 1 + dj:bj1 - 1 + dj]
                nc.vector.tensor_add(out=dst, in0=dst, in1=src)

        # ---- centroid update: cent1 = num / den_s
        rden_s = singles.tile([1, NS], f32)
        nc.vector.reciprocal(rden_s[:], num_sb[C:CH, :])
        bcp = pmisc.tile([CH, NS], f32, tag="bc")
        nc.tensor.matmul(bcp[:], ones1x[:], rden_s[:], start=True, stop=True)
        nc.vector.tensor_mul(cent1[:], num_sb[:], bcp[:])
        build_centP(1, cent1[:], 1.0)

        # ---- iteration 1: affinity -> A9 -> DRAM
        for u in range(NB):
            dot = pdot.tile([128, 32, 9], f32, tag="dot")
            for c in range(32):
                bj, h = c // 2, c % 2
                nc.tensor.matmul(
                    dot[:, c, :], chunk_ap(u, bj, h),
                    centP[1][:].rearrange("c (a b) -> c a b", b=18)[:, u:u + 3, bj:bj + 3],
                    start=True, stop=True)
            e = ep.tile([128, 32, 9], f16, tag="e")
            nc.scalar.activation(e[:], dot[:], mybir.ActivationFunctionType.Exp)
            den = ep.tile([128, 32], f32, tag="den")
            nc.vector.reduce_sum(den[:], e[:], axis=X)
            # 255/den so e*rden is the uint8 code value directly
            nc.vector.tensor_scalar_mul(den[:], den[:], 1.0 / 255.0)
            rden = ep.tile([128, 32], f32, tag="rden")
            nc.vector.reciprocal(rden[:], den[:])
            rd = rden[:]
            rden_bc = bass.AP(tensor=rd.tensor, offset=rd.offset,
                              ap=[rd.ap[0], rd.ap[1], [0, 9]])
            a9 = ep.tile([128, 32, 9], f16, tag="a9")
            nc.vector.tensor_mul(a9[:], e[:], rden_bc)
            a9u = ep.tile([128, 32, 9], u8, tag="a9u")
            nc.vector.tensor_scalar_add(a9u[:], a9[:], 0.5)
            nc.sync.dma_start(out=out9[u], in_=a9u[:].rearrange("p a b -> p (a b)"))

    nc.finalize()
    return nc


_nc = None


def _get_nc():
    global _nc
    if _nc is None:
        _nc = _build_nc()
    return _nc


def host_prep_one(xb):
    """xb (64, 256, 256) f32 -> chunk-major 12-bit planes (a int8, rp uint8).
    chunk (u, bj, h) = image rows 16u+8h..+8, cols 16bj..+16; within-chunk
    pixel p = 16*ii + jj."""
    # (C, u, h, ii, bj, jj) -> (C, u, bj, h, ii, jj)
    xr = xb.reshape(C, NB, 2, 8, NB, SH).transpose(0, 1, 4, 2, 3, 5)
    xc = np.ascontiguousarray(xr, dtype=np.float32).reshape(C, PIX)
    q = np.rint(xc * (1.0 / S10)).astype(np.int16)
    np.clip(q, -511, 511, out=q)
    a = (q >> 2).astype(np.int8)
    r = (q & 3).astype(np.uint8)
    Q4 = PIX // 4
    rp = (r[:, 0:Q4] | (r[:, Q4:2 * Q4] << 2)
          | (r[:, 2 * Q4:3 * Q4] << 4) | (r[:, 3 * Q4:] << 6))
    return a, rp


def host_prep(x):
    xf = np.asarray(x, dtype=np.float32)
    maps = []
    for b in range(B):
        a, rp = host_prep_one(xf[b])
        maps.append({"xs_a": a, "xs_r": rp})
    return maps


def host_reconstruct(outs):
    """outs: list of 4 per-core out9 (16, 128, 288) fp16 -> dense (4,256,65536) f32."""
    a9 = np.stack(outs).astype(np.float32)            # (B, u16, p128, ck288)
    if outs[0].dtype == np.uint8:
        a9 *= 1.0 / 255.0
    a9 = a9.reshape(B, NB, 8, SH, NB, 2, 9)           # (b, u, ii, jj, bj, h, k)
    a9 = a9.transpose(0, 1, 5, 2, 4, 3, 6)            # (b, u, h, ii, bj, jj, k)
    a9 = np.ascontiguousarray(a9).reshape(B, H, W, 9)
    dense = np.zeros((B, NB, NB, NB, SH, NB, SH), dtype=np.float32)
    # dims: (b, si, sj, bi, ii16, bj, jj) ; i = 16*bi + ii, j = 16*bj + jj
    src = a9.reshape(B, NB, SH, NB, SH, 9)            # (b, bi, ii, bj, jj, k)
    for k in range(9):
        di, dj = k // 3 - 1, k % 3 - 1
        b0, b1 = max(0, -di), NB - max(0, di)
        c0, c1 = max(0, -dj), NB - max(0, dj)
        bi = np.arange(b0, b1)
        bj = np.arange(c0, c1)
        for b in range(B):
            # advanced indices at dims 0,1,2 (+slice at 3) -> result dims
            # lead with the broadcasted (bi, bj) index shape
            dense[b][bi[:, None] + di, bj[None, :] + dj, bi[:, None], :, bj[None, :], :] = \
                src[b, b0:b1, :, c0:c1, :, k].transpose(0, 2, 1, 3)
    return dense.reshape(B, NS, PIX)


_exec = None


def _get_exec():
    """Cached jitted SPMD executable.  The stock run_bass_via_pjrt rebuilds
    jax.jit every call, forcing a retrace per kernel() invocation; this
    builds the sharded callable once and reuses it."""
    global _exec
    if _exec is not None:
        return _exec
    import jax
    from jax.experimental.shard_map import shard_map
    from jax.sharding import Mesh, PartitionSpec
    from concourse import bass2jax
    import concourse.mybir as mybir

    bass2jax.install_neuronx_cc_hook()
    nc = _get_nc()
    partition_name = nc.partition_id_tensor.name if nc.partition_id_tensor else None
    in_names, out_names, out_avals = [], [], []
    for alloc in nc.m.functions[0].allocations:
        if not isinstance(alloc, mybir.MemoryLocationSet):
            continue
        name = alloc.memorylocations[0].name
        if alloc.kind == "ExternalInput":
            if name != partition_name:
                in_names.append(name)
        elif alloc.kind == "ExternalOutput":
            out_names.append(name)
            out_avals.append(jax.core.ShapedArray(
                tuple(alloc.tensor_shape), mybir.dt.np(alloc.dtype)))
    n_params = len(in_names)
    all_names = in_names + out_names
    if partition_name is not None:
        all_names = all_names + [partition_name]
    donate = tuple(range(n_params, n_params + len(out_names)))

    def _body(*args):
        operands = list(args)
        if partition_name is not None:
            operands.append(bass2jax.partition_id_tensor())
        return tuple(bass2jax._bass_exec_p.bind(
            *operands,
            out_avals=tuple(out_avals),
            in_names=tuple(all_names),
            out_names=tuple(out_names),
            lowering_input_output_aliases=(),
            sim_require_finite=True,
            sim_require_nnan=True,
            nc=nc,
        ))

    devices = jax.devices()[:B]
    mesh = Mesh(np.asarray(devices), ("core",))
    specs = (PartitionSpec("core"),)
    sharded = jax.jit(
        shard_map(_body, mesh=mesh,
                  in_specs=specs * (n_params + len(out_names)),
                  out_specs=specs * len(out_names), check_rep=False),
        donate_argnums=donate, keep_unused=True)
    _exec = (sharded, in_names, out_names, out_avals, mesh)
    return _exec


_prev_out = None


def kernel(x, stoken):
    global _prev_out
    assert int(stoken) == SH
    import jax
    from jax.sharding import NamedSharding, PartitionSpec
    sharded, in_names, out_names, out_avals, mesh = _get_exec()
    xf = np.asarray(x, dtype=np.float32)
    devices = jax.devices()[:B]
    # sequential per-batch quantize + async put: batch b+1 quantizes on the
    # host while batch b streams over the tunnel (threaded variants measure
    # slower — they serialize the first put behind all quantization)
    shards = {n: [] for n in in_names}
    for b in range(B):
        a, rp = host_prep_one(xf[b])
        shards["xs_a"].append(jax.device_put(a, devices[b]))
        shards["xs_r"].append(jax.device_put(rp, devices[b]))
    gl = []
    for n in in_names:
        per = shards[n]
        gshape = (B * per[0].shape[0], *per[0].shape[1:])
        gl.append(jax.make_array_from_single_device_arrays(
            gshape, NamedSharding(mesh, PartitionSpec("core")), per))
    if _prev_out is None:
        # first call: host zeros get uploaded as the donated output buffer
        outbufs = [np.zeros((B * a.shape[0], *a.shape[1:]), a.dtype)
                   for a in out_avals]
    else:
        # donate last call's device-resident outputs (fully overwritten by
        # the kernel) — avoids re-uploading 9.4MB of zeros over the tunnel
        outbufs = _prev_out
    out_arrs = sharded(*gl, *outbufs)
    o = np.asarray(out_arrs[0]).reshape(B, *out_avals[0].shape)
    _prev_out = list(out_arrs)
    return host_reconstruct([o[b] for b in range(B)])


# revision 41
# speedup vs baseline: 1.2181x; 1.2181x over previous
"""GenSP superpixel affinity for trn2 — Bass kernel, 4 cores batch-parallel.

Math (exact vs reference, not approximate):
- M_COEF=0: the two appended grid channels are identically zero -> dropped.
- Softmax over the 9 candidate superpixels: the per-pixel f2 term cancels
  inside softmax, so logits_k = 2*f.c_k - |c_k|^2.  Computed per 16x16
  pixel block (all 256 pixels of a block share the same 9 candidates) via
  a matmul with an appended constant channel:
      feats' = [f; 1]  (65 ch),  cent'_k = [2*c_k; -|c_k|^2]
      logits = feats'^T @ cent'.
- Invalid (border) candidates get cent' = [0; -30] -> exp(logit) ~ 1e-13,
  and the host drops them entirely when scattering, so they contribute 0.
- The dense (B, 256, 65536) output is 96.5% zeros: the device only computes
  the 9 nonzero values per pixel (A9); the host scatters them into the
  dense array.  This cuts device->host traffic ~50x (the axon tunnel at
  ~40 MB/s dominates wall clock) and kills the dense HBM write.

Device layout per core (one full batch image per core, cores 0-3):
- input  xs   (65, 65536) fp16: 64 feature rows + ones row (host-baked).
- output out9 (16, 128, 288) fp16: [block-row u][pixel-in-chunk][chunk c, k]
  chunk c = 2*bj + h (h = 8-pixel-row half of block (u, bj)),
  pixel p = 16*ii + jj (ii = image row within half, jj = col within block),
  k = 3*di + dj over the 3x3 candidate neighborhood (reference order).
- iter 0: affinity A0 for all pixels + centroid update sums via
  TensorE-transposed feature chunks; iter 1: affinity -> A9 -> DRAM.
"""

import numpy as np
from contextlib import ExitStack

B, C, H, W = 4, 64, 256, 256
SH = 16
NB = 16            # blocks per side
NS = NB * NB       # 256 superpixels
PIX = H * W        # 65536
CH = C + 1         # 65: features + ones row
NEG = -30.0        # border-candidate bias: exp(-30) ~ 9e-14 ~ 0

F16 = np.float16
# 10-bit fixed-point input quantization: x ~ S10 * (4*a + r), a int8, r uint2
# (4 packed per byte).  5/8 the upload bytes of fp16; rel_l2 ~4.8e-3 vs the
# 2e-2 gate (int8 alone measured 0.019, 12-bit 1.3e-3).
S10 = 5.6 / 511.0


def _build_nc():
    import concourse.bass as bass
    import concourse.bacc as bacc
    import concourse.tile as tile
    import concourse.mybir as mybir
    from concourse.masks import make_identity

    f16 = mybir.dt.float16
    f32 = mybir.dt.float32
    X = mybir.AxisListType.X

    # Bacc (not Bass): its finalize() runs move_matmul_waits_to_ldweights +
    # generate_event_semaphores, without which walrus rejects instructions
    # that accumulated >1 semaphore wait ("Too many sync wait commands").
    nc = bacc.Bacc("TRN2")
    i8 = mybir.dt.int8
    u8 = mybir.dt.uint8
    # Inputs are chunk-major (host-prearranged): free index = cidx*128 +
    # (16*ii+jj) with cidx = (u*16 + bj)*2 + h, so every matmul stationary
    # operand is a contiguous (65, 128) slice (walrus: stationary AP must be
    # 1-D free).  xs_a = int8 coarse plane; xs_r = uint2 residuals, crumb c
    # of byte j holds pixel c*PIX/4 + j.
    xs_a = nc.dram_tensor("xs_a", (C, PIX), i8, kind="ExternalInput")
    xs_r = nc.dram_tensor("xs_r", (C, PIX // 4), u8, kind="ExternalInput")
    # output quantized to uint8 (A*255): halves d2h bytes; rel_l2 7e-3
    out9 = nc.dram_tensor("out9", (NB, 128, 288), u8, kind="ExternalOutput")

    with ExitStack() as ctx:
        tc = ctx.enter_context(tile.TileContext(nc))
        singles = ctx.enter_context(tc.tile_pool(name="singles", bufs=1))
        ep = ctx.enter_context(tc.tile_pool(name="ep", bufs=3))
        ft = ctx.enter_context(tc.tile_pool(name="ft", bufs=6))
        pdot = ctx.enter_context(tc.tile_pool(name="pdot", bufs=2, space="PSUM"))
        ptr = ctx.enter_context(tc.tile_pool(name="ptr", bufs=2, space="PSUM"))
        pupd = ctx.enter_context(tc.tile_pool(name="pupd", bufs=2, space="PSUM"))
        pmisc = ctx.enter_context(tc.tile_pool(name="pmisc", bufs=1, space="PSUM"))

        feats = singles.tile([CH, PIX], f16)
        with tc.tile_pool(name="dq", bufs=2) as dq:
            # coarse plane: plain DMA + DVE convert-scale (the SWDGE
            # cast-during-DMA path is far slower); quarters bound SBUF use
            QS = PIX // 4
            for qq in range(4):
                a_t = dq.tile([C, QS], i8, tag="ast", bufs=1)
                nc.sync.dma_start(out=a_t[:], in_=xs_a[:, qq * QS:(qq + 1) * QS])
                nc.vector.tensor_scalar_mul(
                    feats[0:C, qq * QS:(qq + 1) * QS], a_t[:], 4.0 * S10)
            # 2-bit residuals, in segments to bound SBUF scratch
            SEG = PIX // 16
            for s in range(4):
                rp = dq.tile([C, SEG], u8, tag="rp")
                nc.sync.dma_start(out=rp[:], in_=xs_r[:, s * SEG:(s + 1) * SEG])
                for cr in range(4):
                    rn = dq.tile([C, SEG], u8, tag="rn")
                    if cr == 0:
                        nc.vector.tensor_scalar(
                            rn[:], rp[:], 3, None,
                            op0=mybir.AluOpType.bitwise_and)
                    elif cr == 3:
                        nc.vector.tensor_scalar(
                            rn[:], rp[:], 6, None,
                            op0=mybir.AluOpType.logical_shift_right)
                    else:
                        nc.vector.tensor_scalar(
                            rn[:], rp[:], 2 * cr, 3,
                            op0=mybir.AluOpType.logical_shift_right,
                            op1=mybir.AluOpType.bitwise_and)
                    rsc = dq.tile([C, SEG], f16, tag="rsc")
                    nc.vector.tensor_scalar_mul(rsc[:], rn[:], S10)
                    p0 = cr * (PIX // 4) + s * SEG
                    nc.vector.tensor_add(out=feats[0:C, p0:p0 + SEG],
                                         in0=feats[0:C, p0:p0 + SEG], in1=rsc[:])
        # two memsets: a single one gets AP-flattened to 65536 elements,
        # which overflows the 16-bit num_elem ISA field
        nc.vector.memset(feats[C:CH, 0:PIX // 2], 1.0)
        nc.vector.memset(feats[C:CH, PIX // 2:PIX], 1.0)
        feats_v = feats[:].rearrange("c (n p) -> c n p", p=128)  # (65, 512, 128)

        id65 = singles.tile([CH, CH], f16)
        make_identity(nc, id65[:])
        ones64 = singles.tile([C, 1], f32)
        nc.vector.memset(ones64[:], 1.0)
        ones1x = singles.tile([1, CH], f32)
        nc.vector.memset(ones1x[:], 1.0)

        num_sb = singles.tile([CH, NS], f32)
        nc.vector.memset(num_sb[:], 0.0)
        blocksum = singles.tile([C, NS], f32)
        cent1 = singles.tile([CH, NS], f32)
        sqc = singles.tile([C, NS], f32)
        centP = [singles.tile([CH, 18 * 18], f16, tag=f"centP{i}", name=f"centP{i}")
                 for i in range(2)]

        def chunk_ap(u, bj, h):
            # (65, 128) stationary: pixels of half h of block (u, bj)
            return feats_v[:, ((u * NB + bj) * 2 + h), :]

        # ---- init centroids: block sums via two DVE reduces
        rs1 = singles.tile([C, 2 * NS], f32)
        nc.vector.reduce_sum(rs1[:], feats_v[0:C], axis=X)   # per-chunk sums
        nc.vector.reduce_sum(blocksum[:].rearrange("c (a b) -> c a b", b=NB),
                             rs1[:].rearrange("c (n h) -> c n h", h=2), axis=X)

        def build_centP(idx, src, scale):
            # centP rows 0..63 = 2*scale*src (interior), row 64 = -scale^2*|src|^2
            cp = centP[idx]
            cpv = cp[:].rearrange("c (a b) -> c a b", b=18)
            nc.vector.memset(cp[0:C, :], 0.0)
            nc.vector.memset(cp[C:CH, :], NEG)
            nc.vector.tensor_scalar_mul(
                cpv[0:C, 1:17, 1:17],
                src[0:C, :].rearrange("c (a b) -> c a b", b=NB), 2.0 * scale)
            nc.vector.tensor_mul(sqc[:], src[0:C, :], src[0:C, :])
            c2p = pmisc.tile([1, NS], f32, tag="c2")
            nc.tensor.matmul(c2p[:], ones64[:], sqc[:], start=True, stop=True)
            nc.vector.tensor_scalar_mul(
                cpv[C:CH, 1:17, 1:17],
                c2p[:].rearrange("c (a b) -> c a b", b=NB), -(scale * scale))

        build_centP(0, blocksum[:], 1.0 / 256.0)

        # ---- iteration 0: affinity + update sums
        for u in range(NB):
            dot = pdot.tile([128, 32, 9], f32, tag="dot")
            for c in range(32):
                bj, h = c // 2, c % 2
                nc.tensor.matmul(
                    dot[:, c, :], chunk_ap(u, bj, h),
                    centP[0][:].rearrange("c (a b) -> c a b", b=18)[:, u:u + 3, bj:bj + 3],
                    start=True, stop=True)
            e = ep.tile([128, 32, 9], f16, tag="e")
            nc.scalar.activation(e[:], dot[:], mybir.ActivationFunctionType.Exp)
            den = ep.tile([128, 32], f32, tag="den")
            nc.vector.reduce_sum(den[:], e[:], axis=X)
            rden = ep.tile([128, 32], f32, tag="rden")
            nc.vector.reciprocal(rden[:], den[:])
            rd = rden[:]
            rden_bc = bass.AP(tensor=rd.tensor, offset=rd.offset,
                              ap=[rd.ap[0], rd.ap[1], [0, 9]])
            a0 = ep.tile([128, 32, 9], f16, tag="a0")
            nc.vector.tensor_mul(a0[:], e[:], rden_bc)

            upd = pupd.tile([CH, NB, 9], f32, tag="upd")
            for c in range(32):
                bj, h = c // 2, c % 2
                tr = ptr.tile([128, CH], f16, tag="tr")
                nc.tensor.transpose(tr[:], chunk_ap(u, bj, h), id65[:])
                ftc = ft.tile([128, CH], f16, tag="ftc")
                nc.vector.tensor_copy(out=ftc[:], in_=tr[:])
                nc.tensor.matmul(upd[:, bj, :], ftc[:], a0[:, c, :],
                                 start=(h == 0), stop=(h == 1))
            updv = upd[:].rearrange("s b (x y) -> s b x y", y=3)
            for dj in range(3):
                di0, di1 = (1 if u == 0 else 0), (2 if u == NB - 1 else 3)
                bj0, bj1 = (1 if dj == 0 else 0), (NB - 1 if dj == 2 else NB)
                src = updv[:, bj0:bj1, di0:di1, dj].rearrange("s b d -> s d b")
                dst = num_sb[:].rearrange("s (a b) -> s a b", b=NB)[
                    :, u - 1 + di0:u - 1 + di1, bj0 - 1 + dj:bj1 - 1 + dj]
                nc.vector.tensor_add(out=dst, in0=dst, in1=src)

        # ---- centroid update: cent1 = num / den_s
        rden_s = singles.tile([1, NS], f32)
        nc.vector.reciprocal(rden_s[:], num_sb[C:CH, :])
        bcp = pmisc.tile([CH, NS], f32, tag="bc")
        nc.tensor.matmul(bcp[:], ones1x[:], rden_s[:], start=True, stop=True)
        nc.vector.tensor_mul(cent1[:], num_sb[:], bcp[:])
        build_centP(1, cent1[:], 1.0)

        # ---- iteration 1: affinity -> A9 -> DRAM
        for u in range(NB):
            dot = pdot.tile([128, 32, 9], f32, tag="dot")
            for c in range(32):
                bj, h = c // 2, c % 2
                nc.tensor.matmul(
                    dot[:, c, :], chunk_ap(u, bj, h),
                    centP[1][:].rearrange("c (a b) -> c a b", b=18)[:, u:u + 3, bj:bj + 3],
                    start=True, stop=True)
            e = ep.tile([128, 32, 9], f16, tag="e")
            nc.scalar.activation(e[:], dot[:], mybir.ActivationFunctionType.Exp)
            den = ep.tile([128, 32], f32, tag="den")
            nc.vector.reduce_sum(den[:], e[:], axis=X)
            # 255/den so e*rden is the uint8 code value directly
            nc.vector.tensor_scalar_mul(den[:], den[:], 1.0 / 255.0)
            rden = ep.tile([128, 32], f32, tag="rden")
            nc.vector.reciprocal(rden[:], den[:])
            rd = rden[:]
            rden_bc = bass.AP(tensor=rd.tensor, offset=rd.offset,
                              ap=[rd.ap[0], rd.ap[1], [0, 9]])
            a9 = ep.tile([128, 32, 9], f16, tag="a9")
            nc.vector.tensor_mul(a9[:], e[:], rden_bc)
            a9u = ep.tile([128, 32, 9], u8, tag="a9u")
            # HW float->uint8 conversion rounds to nearest (sim truncates;
            # trust HW — adding 0.5 here measured a half-code bias on HW)
            nc.vector.tensor_copy(out=a9u[:], in_=a9[:])
            nc.sync.dma_start(out=out9[u], in_=a9u[:].rearrange("p a b -> p (a b)"))

    nc.finalize()
    return nc


_nc = None


def _get_nc():
    global _nc
    if _nc is None:
        _nc = _build_nc()
    return _nc


def host_prep_one(xb):
    """xb (64, 256, 256) f32 -> chunk-major 12-bit planes (a int8, rp uint8).
    chunk (u, bj, h) = image rows 16u+8h..+8, cols 16bj..+16; within-chunk
    pixel p = 16*ii + jj."""
    # (C, u, h, ii, bj, jj) -> (C, u, bj, h, ii, jj)
    xr = xb.reshape(C, NB, 2, 8, NB, SH).transpose(0, 1, 4, 2, 3, 5)
    xc = np.ascontiguousarray(xr, dtype=np.float32).reshape(C, PIX)
    q = np.rint(xc * (1.0 / S10)).astype(np.int16)
    np.clip(q, -511, 511, out=q)
    a = (q >> 2).astype(np.int8)
    r = (q & 3).astype(np.uint8)
    Q4 = PIX // 4
    rp = (r[:, 0:Q4] | (r[:, Q4:2 * Q4] << 2)
          | (r[:, 2 * Q4:3 * Q4] << 4) | (r[:, 3 * Q4:] << 6))
    return a, rp


def host_prep(x):
    xf = np.asarray(x, dtype=np.float32)
    maps = []
    for b in range(B):
        a, rp = host_prep_one(xf[b])
        maps.append({"xs_a": a, "xs_r": rp})
    return maps


def host_reconstruct(outs):
    """outs: list of 4 per-core out9 (16, 128, 288) fp16 -> dense (4,256,65536) f32."""
    a9 = np.stack(outs).astype(np.float32)            # (B, u16, p128, ck288)
    if outs[0].dtype == np.uint8:
        a9 *= 1.0 / 255.0
    a9 = a9.reshape(B, NB, 8, SH, NB, 2, 9)           # (b, u, ii, jj, bj, h, k)
    a9 = a9.transpose(0, 1, 5, 2, 4, 3, 6)            # (b, u, h, ii, bj, jj, k)
    a9 = np.ascontiguousarray(a9).reshape(B, H, W, 9)
    dense = np.zeros((B, NB, NB, NB, SH, NB, SH), dtype=np.float32)
    # dims: (b, si, sj, bi, ii16, bj, jj) ; i = 16*bi + ii, j = 16*bj + jj
    src = a9.reshape(B, NB, SH, NB, SH, 9)            # (b, bi, ii, bj, jj, k)
    for k in range(9):
        di, dj = k // 3 - 1, k % 3 - 1
        b0, b1 = max(0, -di), NB - max(0, di)
        c0, c1 = max(0, -dj), NB - max(0, dj)
        bi = np.arange(b0, b1)
        bj = np.arange(c0, c1)
        for b in range(B):
            # advanced indices at dims 0,1,2 (+slice at 3) -> result dims
            # lead with the broadcasted (bi, bj) index shape
            dense[b][bi[:, None] + di, bj[None, :] + dj, bi[:, None], :, bj[None, :], :] = \
                src[b, b0:b1, :, c0:c1, :, k].transpose(0, 2, 1, 3)
    return dense.reshape(B, NS, PIX)


_exec = None


def _get_exec():
    """Cached jitted SPMD executable.  The stock run_bass_via_pjrt rebuilds
    jax.jit every call, forcing a retrace per kernel() invocation; this
    builds the sharded callable once and reuses it."""
    global _exec
    if _exec is not None:
        return _exec
    import jax
    from jax.experimental.shard_map import shard_map
    from jax.sharding import Mesh, PartitionSpec
    from concourse import bass2jax
    import concourse.mybir as mybir

    bass2jax.install_neuronx_cc_hook()
    nc = _get_nc()
    partition_name = nc.partition_id_tensor.name if nc.partition_id_tensor else None
    in_names, out_names, out_avals = [], [], []
    for alloc in nc.m.functions[0].allocations:
        if not isinstance(alloc, mybir.MemoryLocationSet):
            continue
        name = alloc.memorylocations[0].name
        if alloc.kind == "ExternalInput":
            if name != partition_name:
                in_names.append(name)
        elif alloc.kind == "ExternalOutput":
            out_names.append(name)
            out_avals.append(jax.core.ShapedArray(
                tuple(alloc.tensor_shape), mybir.dt.np(alloc.dtype)))
    n_params = len(in_names)
    all_names = in_names + out_names
    if partition_name is not None:
        all_names = all_names + [partition_name]
    donate = tuple(range(n_params, n_params + len(out_names)))

    def _body(*args):
        operands = list(args)
        if partition_name is not None:
            operands.append(bass2jax.partition_id_tensor())
        return tuple(bass2jax._bass_exec_p.bind(
            *operands,
            out_avals=tuple(out_avals),
            in_names=tuple(all_names),
            out_names=tuple(out_names),
            lowering_input_output_aliases=(),
            sim_require_finite=True,
            sim_require_nnan=True,
            nc=nc,
        ))

    devices = jax.devices()[:B]
    mesh = Mesh(np.asarray(devices), ("core",))
    specs = (PartitionSpec("core"),)
    sharded = jax.jit(
        shard_map(_body, mesh=mesh,
                  in_specs=specs * (n_params + len(out_names)),
                  out_specs=specs * len(out_names), check_rep=False),
        donate_argnums=donate, keep_unused=True)
    _exec = (sharded, in_names, out_names, out_avals, mesh)
    return _exec


_prev_out = None


def kernel(x, stoken):
    global _prev_out
    assert int(stoken) == SH
    import jax
    from jax.sharding import NamedSharding, PartitionSpec
    sharded, in_names, out_names, out_avals, mesh = _get_exec()
    xf = np.asarray(x, dtype=np.float32)
    devices = jax.devices()[:B]
    # sequential per-batch quantize + async put: batch b+1 quantizes on the
    # host while batch b streams over the tunnel (threaded variants measure
    # slower — they serialize the first put behind all quantization)
    shards = {n: [] for n in in_names}
    for b in range(B):
        a, rp = host_prep_one(xf[b])
        shards["xs_a"].append(jax.device_put(a, devices[b]))
        shards["xs_r"].append(jax.device_put(rp, devices[b]))
    gl = []
    for n in in_names:
        per = shards[n]
        gshape = (B * per[0].shape[0], *per[0].shape[1:])
        gl.append(jax.make_array_from_single_device_arrays(
            gshape, NamedSharding(mesh, PartitionSpec("core")), per))
    if _prev_out is None:
        # first call: host zeros get uploaded as the donated output buffer
        outbufs = [np.zeros((B * a.shape[0], *a.shape[1:]), a.dtype)
                   for a in out_avals]
    else:
        # donate last call's device-resident outputs (fully overwritten by
        # the kernel) — avoids re-uploading 9.4MB of zeros over the tunnel
        outbufs = _prev_out
    out_arrs = sharded(*gl, *outbufs)
    o = np.asarray(out_arrs[0]).reshape(B, *out_avals[0].shape)
    _prev_out = list(out_arrs)
    return host_reconstruct([o[b] for b in range(B)])
